# revision 25
# baseline (speedup 1.0000x reference)
"""AnyNet _build_volume_2d cost-volume kernel for 8 TRN2 NeuronCores.

Strategy: data-parallel over batch B=8 (one sample per core).  The bilinear
grid-sample gather runs on the SWDGE `dma_gather` path: one 96B descriptor
per output point fetches a 3-column x 2-row-plane x 8-channel fp16 window of
a host-prepacked table, so all 4 bilinear taps for all 8 channels arrive
with a single int16 index.  Interpolation weights are hat functions
evaluated on-device; zero padding baked into the table plus the hat
formulation reproduces grid_sample's zeros padding exactly.  The masked-inf
channel sum is computed as max(sum_c |diff_c|, +-inf) where the +-inf flag
comes from a u8 mask reduce (avoids 0*inf NaNs).

Table layout (built on host from feat_r), fp16:
  T2[yi, w, d, pl, c] = padded_feat[pl][yi][2w + d][c]   (d in [0,3), rest pad)
  padded_feat[0][yi][xc][c] = feat_r[c, yi-1, xc-1]   (zeros out of range)
  padded_feat[1][yi][xc][c] = feat_r[c, yi,   xc-1]
Rows are padded to 128 fp16 = 256B (dma_gather stride granule); the gather
fetches the first 48 fp16 (96B).  Gather index = yi*210 + w, w = xi>>1
(2-column phase windows), fits int16.

The wrapped-16 int16 index slab dma_gather requires is built with
TensorEngine permutation matmuls (engines cannot cross partitions), then
replicated to all 8 16-partition groups with a second matmul.
"""

import sys

sys.path.insert(0, "/opt/trn_rl_repo")

import numpy as np

B, C, S, H, W = 8, 8, 16, 128, 416
ROWS, WINS = 129, 210          # table rows (H+1), windows per row (W/2+2)
NTAB = ROWS * WINS             # 27090
RW = 128                       # fp16 per table row (256B stride)
EF = 48                        # fp16 fetched per index (96B: 3 cols x 16)
J = 208                        # points per partition per sub-chunk
NSUB = W // J                  # 2 sub-chunks per s-step
GIDX = 1024                    # indices per gather (hard ucode limit)
NGS = (128 * J) // GIDX        # 26 gathers per sub-chunk

_BUILT = None


def _raw_gather(nc, out_ap, in_ap, idxs_ap, num_idxs, elem_size, elem_step, queue_num=0):
    """dma_gather with elem_size below the wrapper's 256B floor (HW-verified)."""
    from concourse import mybir
    gp = nc.gpsimd
    stride_bytes = elem_step * mybir.dt.size(in_ap.dtype)
    _in_ap = gp.lower_ap_dma(in_ap, for_custom_bir_dma=True)
    return gp.add_instruction(
        mybir.InstDMAGatherAnt(
            name=nc.get_next_instruction_name(),
            ins=[*_in_ap, gp.lower_ap(idxs_ap),
                 gp.lower_val_access(gp.to_reg(num_idxs))],
            outs=[gp.lower_ap(out_ap)],
            transpose=False, num_idxs=num_idxs, elem_size=elem_size,
            stride_bytes_256=stride_bytes // 256, gen_mode=0,
            single_packet=False, queue_num=queue_num,
            sbuf_tokens_per_rank=0, sbuf_free_dim_per_rank=0,
            sbuf_free_dim_pad_per_rank=0, sbuf_byte_offset=0))


def _build_module():
    import concourse.bacc as bacc
    import concourse.tile as tile
    from concourse import mybir

    F32, F16, I16, U8 = (mybir.dt.float32, mybir.dt.float16, mybir.dt.int16,
                         mybir.dt.uint8)
    AF = mybir.ActivationFunctionType
    ALU = mybir.AluOpType
    AX = mybir.AxisListType

    nc = bacc.Bacc("TRN2", target_bir_lowering=False, debug=False,
                   num_devices=8, num_swdge_queues=2)
    t2 = nc.declare_dram_parameter("t2", [NTAB, RW], F16, isOutput=False)
    lut3 = nc.declare_dram_parameter("lut3", [S, 128, W * 3], F32, isOutput=False)
    vmask = nc.declare_dram_parameter("vmask", [S, 128, W * 8], U8, isOutput=False)
    fl_in = nc.declare_dram_parameter("fl", [128, W * 8], F32, isOutput=False)
    iden_in = nc.declare_dram_parameter("iden", [128, 128], F32, isOutput=False)
    rep_in = nc.declare_dram_parameter("rep16", [16, 128], F32, isOutput=False)
    out = nc.declare_dram_parameter("out", [S, 128, W], F32, isOutput=True)

    TWO23 = float(2.0 ** 23)
    BIG = 3.0e38

    with tile.TileContext(nc) as tc:
        with tc.tile_pool(name="const", bufs=1) as cst, \
             tc.tile_pool(name="work", bufs=2) as wk, \
             tc.tile_pool(name="gat", bufs=8) as gp_, \
             tc.tile_pool(name="slb", bufs=3) as slb, \
             tc.tile_pool(name="psel", bufs=1, space="PSUM") as psel, \
             tc.tile_pool(name="prep", bufs=1, space="PSUM") as prep:

            fl = cst.tile([128, W * 8], F32, tag="fl")
            nc.sync.dma_start(out=fl[:], in_=fl_in[:])
            iden = cst.tile([128, 128], F32, tag="iden")
            nc.sync.dma_start(out=iden[:], in_=iden_in[:])
            rep16 = cst.tile([16, 128], F32, tag="rep16")
            nc.sync.dma_start(out=rep16[:], in_=rep_in[:])
            cm1 = cst.tile([128, 1], F32, tag="cm1")
            nc.vector.memset(cm1[:], -1.0)
            cm2 = cst.tile([128, 1], F32, tag="cm2")
            nc.vector.memset(cm2[:], -2.0)

            def build_prep(s, cch):
                j0 = cch * J
                # ---- lut load + coordinate prep ----
                Lt = wk.tile([128, J * 3], F32, tag="Lt", name="Lt")
                nc.sync.dma_start(out=Lt[:], in_=lut3[s][:, j0 * 3:(j0 + J) * 3])
                lv = Lt[:].rearrange("p (j k) -> p j k", k=3)
                gx, gy = lv[:, :, 0], lv[:, :, 1]

                ixc = wk.tile([128, J], F32, tag="ixc", name="ixc")
                nc.scalar.activation(ixc[:], gx, AF.Copy, bias=208.5, scale=208.0)
                iyr = wk.tile([128, J], F32, tag="iyr", name="iyr")
                nc.scalar.activation(iyr[:], gy, AF.Copy, bias=64.5, scale=64.0)

                tcl = wk.tile([128, J], F32, tag="tcl", name="tcl")
                nc.vector.tensor_scalar(out=tcl[:], in0=ixc[:], scalar1=0.0,
                                        scalar2=416.9, op0=ALU.max, op1=ALU.min)
                th = wk.tile([128, J], F32, tag="th", name="th")
                nc.scalar.activation(th[:], tcl[:], AF.Copy, bias=TWO23, scale=0.5)
                rt = wk.tile([128, J], F32, tag="rt", name="rt")
                nc.scalar.activation(rt[:], th[:], AF.Copy, bias=-TWO23, scale=1.0)
                t2_ = wk.tile([128, J], F32, tag="t2_", name="t2_")
                nc.scalar.activation(t2_[:], tcl[:], AF.Copy, bias=0.0, scale=0.5)
                g1 = wk.tile([128, J], F32, tag="g1", name="g1")
                nc.vector.tensor_tensor(out=g1[:], in0=rt[:], in1=t2_[:], op=ALU.is_gt)
                wf = wk.tile([128, J], F32, tag="wf", name="wf")
                nc.vector.tensor_tensor(out=wf[:], in0=rt[:], in1=g1[:], op=ALU.subtract)
                w2 = wk.tile([128, J], F32, tag="w2", name="w2")
                nc.scalar.activation(w2[:], wf[:], AF.Copy, bias=0.0, scale=-2.0)
                u = wk.tile([128, J], F32, tag="u", name="u")
                nc.vector.tensor_tensor(out=u[:], in0=ixc[:], in1=w2[:], op=ALU.add)

                tyc = wk.tile([128, J], F32, tag="tyc", name="tyc")
                nc.vector.tensor_scalar(out=tyc[:], in0=iyr[:], scalar1=0.0,
                                        scalar2=128.9, op0=ALU.max, op1=ALU.min)
                rty = wk.tile([128, J], F32, tag="rty", name="rty")
                nc.vector.tensor_scalar(out=rty[:], in0=tyc[:], scalar1=TWO23,
                                        scalar2=-TWO23, op0=ALU.add, op1=ALU.add)
                gy1 = wk.tile([128, J], F32, tag="gy1", name="gy1")
                nc.vector.tensor_tensor(out=gy1[:], in0=rty[:], in1=tyc[:], op=ALU.is_gt)
                yi = wk.tile([128, J], F32, tag="yi", name="yi")
                nc.vector.tensor_tensor(out=yi[:], in0=rty[:], in1=gy1[:], op=ALU.subtract)
                vv = wk.tile([128, J], F32, tag="vv", name="vv")
                nc.vector.tensor_tensor(out=vv[:], in0=iyr[:], in1=yi[:], op=ALU.subtract)
                y210 = wk.tile([128, J], F32, tag="y210", name="y210")
                nc.scalar.activation(y210[:], yi[:], AF.Copy, bias=0.0, scale=210.0)
                idxf = wk.tile([128, J], F32, tag="idxf", name="idxf")
                nc.vector.tensor_tensor(out=idxf[:], in0=y210[:], in1=wf[:], op=ALU.add)

                # ---- hat weights (fp16) ----
                wx = []
                for d in range(3):
                    a = wk.tile([128, J], F32, tag=f"a{d}", name=f"a{d}")
                    bias = [0.0, cm1[:], cm2[:]][d]
                    nc.scalar.activation(a[:], u[:], AF.Abs, bias=bias, scale=1.0)
                    h = wk.tile([128, J], F16, tag=f"wx{d}", name=f"wx{d}")
                    nc.scalar.activation(h[:], a[:], AF.Relu, bias=1.0, scale=-1.0)
                    wx.append(h)
                wy = []
                for p in range(2):
                    a = wk.tile([128, J], F32, tag=f"ay{p}", name=f"ay{p}")
                    bias = [0.0, cm1[:]][p]
                    nc.scalar.activation(a[:], vv[:], AF.Abs, bias=bias, scale=1.0)
                    h = wk.tile([128, J], F16, tag=f"wy{p}", name=f"wy{p}")
                    nc.scalar.activation(h[:], a[:], AF.Relu, bias=1.0, scale=-1.0)
                    wy.append(h)
                W6 = slb.tile([128, J * 6], F16, tag="W6", name="W6")
                w6v = W6[:].rearrange("p (j k) -> p j k", k=6)
                for d in range(3):
                    for p in range(2):
                        nc.vector.tensor_tensor(out=w6v[:, :, d * 2 + p],
                                                in0=wx[d][:], in1=wy[p][:], op=ALU.mult)

                # ---- index slab via PE permutation; replicate via DMA ----
                s16 = slb.tile([16, J * 8], I16, tag="s16", name="s16")
                sv = s16[:].rearrange("q (j g) -> q j g", g=8)
                sel_ps = [psel.tile([16, 512], F32, tag=f"sel{h_}", name=f"sel{h_}") for h_ in range(4)]
                for g in range(8):
                    ps, off = sel_ps[g // 2], (g % 2) * J
                    nc.tensor.matmul(ps[:, off:off + J], iden[:, 16 * g:16 * g + 16],
                                     idxf[:], start=True, stop=True)
                for h_ in range(4):
                    pv_ = sel_ps[h_][:, 0:2 * J].rearrange("q (g j) -> q j g", g=2)
                    nc.scalar.copy(sv[:, :, 2 * h_:2 * h_ + 2], pv_)
                slab16 = slb.tile([128, J * 8], I16, tag="slab16", name="slab16")
                for g in range(8):
                    nc.sync.dma_start(out=slab16[16 * g:16 * g + 16, :], in_=s16[:])
                return slab16, w6v

            def consume(s, cch, slab16, w6v):
                j0 = cch * J
                warped = wk.tile([128, J * 8], F32, tag="warped", name="warped")
                SL = GIDX // 128   # slots per gather
                for k in range(NGS):
                    G = gp_.tile([128, SL * EF], F16, tag="G", name="G")
                    _raw_gather(nc, G[:].rearrange("p (s e) -> p s e", e=EF),
                                t2[:][:, 0:EF],
                                slab16[:, k * (GIDX // 16):(k + 1) * (GIDX // 16)],
                                GIDX, elem_size=EF, elem_step=RW, queue_num=k % 2)
                    gv = G[:].rearrange("p (j c k) -> p j c k", c=8, k=6)
                    nc.vector.tensor_tensor(
                        out=gv, in0=gv,
                        in1=w6v[:, k * SL:(k + 1) * SL, :]
                            .unsqueeze(2).broadcast_to([128, SL, 8, 6]),
                        op=ALU.mult)
                    nc.vector.tensor_reduce(
                        out=warped[:, k * SL * 8:(k + 1) * SL * 8],
                        in_=gv, axis=AX.X, op=ALU.add)

                # ---- |diff|, channel sum, masked inf ----
                dtl = wk.tile([128, J * 8], F32, tag="dtl", name="dtl")
                nc.vector.tensor_tensor(out=dtl[:], in0=warped[:],
                                        in1=fl[:, j0 * 8:(j0 + J) * 8], op=ALU.subtract)
                sa = wk.tile([128, J], F32, tag="sa", name="sa")
                nc.vector.tensor_reduce(out=sa[:],
                                        in_=dtl[:].rearrange("p (j c) -> p j c", c=8),
                                        axis=AX.X, op=ALU.add,
                                        apply_absolute_value=True)
                V = wk.tile([128, J * 8], U8, tag="V", name="V")
                nc.sync.dma_start(out=V[:], in_=vmask[s][:, j0 * 8:(j0 + J) * 8])
                nv = wk.tile([128, J], F32, tag="nv", name="nv")
                nc.vector.tensor_reduce(out=nv[:],
                                        in_=V[:].rearrange("p (j c) -> p j c", c=8),
                                        axis=AX.X, op=ALU.add)
                q = wk.tile([128, J], F32, tag="q", name="q")
                nc.vector.tensor_scalar(out=q[:], in0=nv[:], scalar1=-1.0,
                                        scalar2=7.5, op0=ALU.mult, op1=ALU.add)
                nc.vector.tensor_scalar(out=q[:], in0=q[:], scalar1=BIG,
                                        scalar2=BIG, op0=ALU.mult, op1=ALU.mult)
                O = wk.tile([128, J], F32, tag="O", name="O")
                nc.vector.tensor_tensor(out=O[:], in0=sa[:], in1=q[:], op=ALU.max)
                nc.sync.dma_start(out=out[s][:, j0:j0 + J], in_=O[:])

            # 2-deep software pipeline: slab for k+2 built before consuming k
            chunks = [(s, cch) for s in range(S) for cch in range(NSUB)]
            fifo = [build_prep(*chunks[0]), build_prep(*chunks[1])]
            for i, (s, cch) in enumerate(chunks):
                cur = fifo.pop(0)
                if i + 2 < len(chunks):
                    fifo.append(build_prep(*chunks[i + 2]))
                consume(s, cch, *cur)

    nc.compile()
    return nc


def _get_module():
    global _BUILT
    if _BUILT is None:
        _BUILT = _build_module()
    return _BUILT


def _build_t2(fr):
    """fr: [C, H, W] f32 -> [NTAB, RW] fp16 table."""
    t = np.ascontiguousarray(fr.transpose(1, 2, 0))      # [H, W, C]
    P = np.zeros((2, ROWS, 422, 8), np.float32)
    P[0, 1:129, 1:417] = t
    P[1, 0:128, 1:417] = t
    T2 = np.zeros((ROWS, WINS, RW), np.float16)
    view = T2[:, :, :EF].reshape(ROWS, WINS, 8, 3, 2)
    for d in range(3):
        # view[yi, w, c, d, pl] = P[pl, yi, 2w+d, c]
        view[:, :, :, d, :] = P[:, :, d:d + 420:2][:, :, :WINS].transpose(1, 2, 3, 0)
    return T2.reshape(NTAB, RW)


def kernel(feat_l, feat_r, lut, valid_mask, no_steps=16):
    from concourse.bass_utils import run_bass_kernel_spmd

    feat_l = np.asarray(feat_l, dtype=np.float32)
    feat_r = np.asarray(feat_r, dtype=np.float32)
    lut = np.asarray(lut, dtype=np.float32)
    vm = np.asarray(valid_mask)

    nc = _get_module()

    idm = np.eye(128, dtype=np.float32)
    repm = np.zeros((16, 128), np.float32)
    for m in range(128):
        repm[m % 16, m] = 1.0

    in_maps = []
    for b in range(B):
        in_maps.append({
            "t2": _build_t2(feat_r[b]),
            "lut3": np.ascontiguousarray(lut[b].reshape(S, H, W * 3)),
            "vmask": np.ascontiguousarray(
                vm[b].transpose(1, 2, 3, 0).astype(np.uint8).reshape(S, H, W * 8)),
            "fl": np.ascontiguousarray(
                feat_l[b].transpose(1, 2, 0).reshape(H, W * 8)),
            "iden": idm,
            "rep16": repm,
        })

    res = run_bass_kernel_spmd(nc, in_maps, core_ids=list(range(8)))
    outv = np.stack([res.results[b]["out"] for b in range(B)])
    return outv.astype(np.float32)


# revision 26
# speedup vs baseline: 1.3167x; 1.3167x over previous
"""AnyNet _build_volume_2d cost-volume kernel for 8 TRN2 NeuronCores.

Strategy: data-parallel over batch B=8 (one sample per core).  The bilinear
grid-sample gather runs on the SWDGE `dma_gather` path: one 96B descriptor
per output point fetches a 3-column x 2-row-plane x 8-channel fp16 window of
a host-prepacked table, so all 4 bilinear taps for all 8 channels arrive
with a single int16 index.  Interpolation weights are hat functions
evaluated on-device; zero padding baked into the table plus the hat
formulation reproduces grid_sample's zeros padding exactly.  The masked-inf
channel sum is computed as max(sum_c |diff_c|, +-inf) where the +-inf flag
comes from a u8 mask reduce (avoids 0*inf NaNs).

Table layout (built on host from feat_r), fp16:
  T2[yi, w, d, pl, c] = padded_feat[pl][yi][2w + d][c]   (d in [0,3), rest pad)
  padded_feat[0][yi][xc][c] = feat_r[c, yi-1, xc-1]   (zeros out of range)
  padded_feat[1][yi][xc][c] = feat_r[c, yi,   xc-1]
Rows are padded to 128 fp16 = 256B (dma_gather stride granule); the gather
fetches the first 48 fp16 (96B).  Gather index = yi*210 + w, w = xi>>1
(2-column phase windows), fits int16.

The wrapped-16 int16 index slab dma_gather requires is built with
TensorEngine permutation matmuls (engines cannot cross partitions), then
replicated to all 8 16-partition groups with a second matmul.
"""

import sys

sys.path.insert(0, "/opt/trn_rl_repo")

import numpy as np

B, C, S, H, W = 8, 8, 16, 128, 416
ROWS, WINS = 129, 210          # table rows (H+1), windows per row (W/2+2)
NTAB = ROWS * WINS             # 27090
RW = 128                       # fp16 per table row (256B stride)
EF = 48                        # fp16 fetched per index (96B: 3 cols x 16)
J = 208                        # points per partition per sub-chunk
NSUB = W // J                  # 2 sub-chunks per s-step
GIDX = 2048                    # per gather (single_packet=False lifts 1024 cap)
NGS = (128 * J) // GIDX        # 26 gathers per sub-chunk

_BUILT = None


def _raw_gather(nc, out_ap, in_ap, idxs_ap, num_idxs, elem_size, elem_step, queue_num=0):
    """dma_gather with elem_size below the wrapper's 256B floor (HW-verified)."""
    from concourse import mybir
    gp = nc.gpsimd
    stride_bytes = elem_step * mybir.dt.size(in_ap.dtype)
    _in_ap = gp.lower_ap_dma(in_ap, for_custom_bir_dma=True)
    return gp.add_instruction(
        mybir.InstDMAGatherAnt(
            name=nc.get_next_instruction_name(),
            ins=[*_in_ap, gp.lower_ap(idxs_ap),
                 gp.lower_val_access(gp.to_reg(num_idxs))],
            outs=[gp.lower_ap(out_ap)],
            transpose=False, num_idxs=num_idxs, elem_size=elem_size,
            stride_bytes_256=stride_bytes // 256, gen_mode=0,
            single_packet=False, queue_num=queue_num,
            sbuf_tokens_per_rank=0, sbuf_free_dim_per_rank=0,
            sbuf_free_dim_pad_per_rank=0, sbuf_byte_offset=0))


def _build_module():
    import concourse.bacc as bacc
    import concourse.tile as tile
    from concourse import mybir

    F32, F16, I16, U8 = (mybir.dt.float32, mybir.dt.float16, mybir.dt.int16,
                         mybir.dt.uint8)
    AF = mybir.ActivationFunctionType
    ALU = mybir.AluOpType
    AX = mybir.AxisListType

    nc = bacc.Bacc("TRN2", target_bir_lowering=False, debug=False,
                   num_devices=8, num_swdge_queues=2)
    t2 = nc.declare_dram_parameter("t2", [NTAB, RW], F16, isOutput=False)
    lut3 = nc.declare_dram_parameter("lut3", [S, 128, W * 3], F32, isOutput=False)
    vmask = nc.declare_dram_parameter("vmask", [S, 128, W * 8], U8, isOutput=False)
    fl_in = nc.declare_dram_parameter("fl", [128, W * 8], F32, isOutput=False)
    iden_in = nc.declare_dram_parameter("iden", [128, 128], F32, isOutput=False)
    rep_in = nc.declare_dram_parameter("rep16", [16, 128], F32, isOutput=False)
    out = nc.declare_dram_parameter("out", [S, 128, W], F32, isOutput=True)

    TWO23 = float(2.0 ** 23)
    BIG = 3.0e38

    with tile.TileContext(nc) as tc:
        with tc.tile_pool(name="const", bufs=1) as cst, \
             tc.tile_pool(name="work", bufs=2) as wk, \
             tc.tile_pool(name="gat", bufs=10) as gp_, \
             tc.tile_pool(name="slb", bufs=3) as slb, \
             tc.tile_pool(name="psel", bufs=1, space="PSUM") as psel, \
             tc.tile_pool(name="prep", bufs=1, space="PSUM") as prep:

            fl = cst.tile([128, W * 8], F32, tag="fl")
            nc.sync.dma_start(out=fl[:], in_=fl_in[:])
            iden = cst.tile([128, 128], F32, tag="iden")
            nc.sync.dma_start(out=iden[:], in_=iden_in[:])
            rep16 = cst.tile([16, 128], F32, tag="rep16")
            nc.sync.dma_start(out=rep16[:], in_=rep_in[:])
            cm1 = cst.tile([128, 1], F32, tag="cm1")
            nc.vector.memset(cm1[:], -1.0)
            cm2 = cst.tile([128, 1], F32, tag="cm2")
            nc.vector.memset(cm2[:], -2.0)

            def build_prep(s, cch):
                j0 = cch * J
                # ---- lut load + coordinate prep ----
                Lt = wk.tile([128, J * 3], F32, tag="Lt", name="Lt")
                nc.sync.dma_start(out=Lt[:], in_=lut3[s][:, j0 * 3:(j0 + J) * 3])
                lv = Lt[:].rearrange("p (j k) -> p j k", k=3)
                gx, gy = lv[:, :, 0], lv[:, :, 1]

                ixc = wk.tile([128, J], F32, tag="ixc", name="ixc")
                nc.scalar.activation(ixc[:], gx, AF.Copy, bias=208.5, scale=208.0)
                iyr = wk.tile([128, J], F32, tag="iyr", name="iyr")
                nc.scalar.activation(iyr[:], gy, AF.Copy, bias=64.5, scale=64.0)

                tcl = wk.tile([128, J], F32, tag="tcl", name="tcl")
                nc.vector.tensor_scalar(out=tcl[:], in0=ixc[:], scalar1=0.0,
                                        scalar2=416.9, op0=ALU.max, op1=ALU.min)
                th = wk.tile([128, J], F32, tag="th", name="th")
                nc.scalar.activation(th[:], tcl[:], AF.Copy, bias=TWO23, scale=0.5)
                rt = wk.tile([128, J], F32, tag="rt", name="rt")
                nc.scalar.activation(rt[:], th[:], AF.Copy, bias=-TWO23, scale=1.0)
                t2_ = wk.tile([128, J], F32, tag="t2_", name="t2_")
                nc.scalar.activation(t2_[:], tcl[:], AF.Copy, bias=0.0, scale=0.5)
                g1 = wk.tile([128, J], F32, tag="g1", name="g1")
                nc.vector.tensor_tensor(out=g1[:], in0=rt[:], in1=t2_[:], op=ALU.is_gt)
                wf = wk.tile([128, J], F32, tag="wf", name="wf")
                nc.vector.tensor_tensor(out=wf[:], in0=rt[:], in1=g1[:], op=ALU.subtract)
                w2 = wk.tile([128, J], F32, tag="w2", name="w2")
                nc.scalar.activation(w2[:], wf[:], AF.Copy, bias=0.0, scale=-2.0)
                u = wk.tile([128, J], F32, tag="u", name="u")
                nc.vector.tensor_tensor(out=u[:], in0=ixc[:], in1=w2[:], op=ALU.add)

                tyc = wk.tile([128, J], F32, tag="tyc", name="tyc")
                nc.vector.tensor_scalar(out=tyc[:], in0=iyr[:], scalar1=0.0,
                                        scalar2=128.9, op0=ALU.max, op1=ALU.min)
                rty = wk.tile([128, J], F32, tag="rty", name="rty")
                nc.vector.tensor_scalar(out=rty[:], in0=tyc[:], scalar1=TWO23,
                                        scalar2=-TWO23, op0=ALU.add, op1=ALU.add)
                gy1 = wk.tile([128, J], F32, tag="gy1", name="gy1")
                nc.vector.tensor_tensor(out=gy1[:], in0=rty[:], in1=tyc[:], op=ALU.is_gt)
                yi = wk.tile([128, J], F32, tag="yi", name="yi")
                nc.vector.tensor_tensor(out=yi[:], in0=rty[:], in1=gy1[:], op=ALU.subtract)
                vv = wk.tile([128, J], F32, tag="vv", name="vv")
                nc.vector.tensor_tensor(out=vv[:], in0=iyr[:], in1=yi[:], op=ALU.subtract)
                y210 = wk.tile([128, J], F32, tag="y210", name="y210")
                nc.scalar.activation(y210[:], yi[:], AF.Copy, bias=0.0, scale=210.0)
                idxf = wk.tile([128, J], F32, tag="idxf", name="idxf")
                nc.vector.tensor_tensor(out=idxf[:], in0=y210[:], in1=wf[:], op=ALU.add)

                # ---- hat weights (fp16) ----
                wx = []
                for d in range(3):
                    a = wk.tile([128, J], F32, tag=f"a{d}", name=f"a{d}")
                    bias = [0.0, cm1[:], cm2[:]][d]
                    nc.scalar.activation(a[:], u[:], AF.Abs, bias=bias, scale=1.0)
                    h = wk.tile([128, J], F16, tag=f"wx{d}", name=f"wx{d}")
                    nc.scalar.activation(h[:], a[:], AF.Relu, bias=1.0, scale=-1.0)
                    wx.append(h)
                wy = []
                for p in range(2):
                    a = wk.tile([128, J], F32, tag=f"ay{p}", name=f"ay{p}")
                    bias = [0.0, cm1[:]][p]
                    nc.scalar.activation(a[:], vv[:], AF.Abs, bias=bias, scale=1.0)
                    h = wk.tile([128, J], F16, tag=f"wy{p}", name=f"wy{p}")
                    nc.scalar.activation(h[:], a[:], AF.Relu, bias=1.0, scale=-1.0)
                    wy.append(h)
                W6 = slb.tile([128, J * 6], F16, tag="W6", name="W6")
                w6v = W6[:].rearrange("p (j k) -> p j k", k=6)
                for d in range(3):
                    for p in range(2):
                        nc.vector.tensor_tensor(out=w6v[:, :, d * 2 + p],
                                                in0=wx[d][:], in1=wy[p][:], op=ALU.mult)

                # ---- index slab via PE permutation; replicate via DMA ----
                s16 = slb.tile([16, J * 8], I16, tag="s16", name="s16")
                sv = s16[:].rearrange("q (j g) -> q j g", g=8)
                sel_ps = [psel.tile([16, 512], F32, tag=f"sel{h_}", name=f"sel{h_}") for h_ in range(4)]
                for g in range(8):
                    ps, off = sel_ps[g // 2], (g % 2) * J
                    nc.tensor.matmul(ps[:, off:off + J], iden[:, 16 * g:16 * g + 16],
                                     idxf[:], start=True, stop=True)
                for h_ in range(4):
                    pv_ = sel_ps[h_][:, 0:2 * J].rearrange("q (g j) -> q j g", g=2)
                    nc.scalar.copy(sv[:, :, 2 * h_:2 * h_ + 2], pv_)
                slab16 = slb.tile([128, J * 8], I16, tag="slab16", name="slab16")
                for g in range(8):
                    nc.sync.dma_start(out=slab16[16 * g:16 * g + 16, :], in_=s16[:])
                return slab16, w6v

            def consume(s, cch, slab16, w6v):
                j0 = cch * J
                warped = wk.tile([128, J * 8], F32, tag="warped", name="warped")
                SL = GIDX // 128   # slots per gather
                for k in range(NGS):
                    G = gp_.tile([128, SL * EF], F16, tag="G", name="G")
                    _raw_gather(nc, G[:].rearrange("p (s e) -> p s e", e=EF),
                                t2[:][:, 0:EF],
                                slab16[:, k * (GIDX // 16):(k + 1) * (GIDX // 16)],
                                GIDX, elem_size=EF, elem_step=RW, queue_num=k % 2)
                    gv = G[:].rearrange("p (j c k) -> p j c k", c=8, k=6)
                    nc.vector.tensor_tensor(
                        out=gv, in0=gv,
                        in1=w6v[:, k * SL:(k + 1) * SL, :]
                            .unsqueeze(2).broadcast_to([128, SL, 8, 6]),
                        op=ALU.mult)
                    nc.vector.tensor_reduce(
                        out=warped[:, k * SL * 8:(k + 1) * SL * 8],
                        in_=gv, axis=AX.X, op=ALU.add)

                # ---- |diff|, channel sum, masked inf ----
                dtl = wk.tile([128, J * 8], F32, tag="dtl", name="dtl")
                nc.vector.tensor_tensor(out=dtl[:], in0=warped[:],
                                        in1=fl[:, j0 * 8:(j0 + J) * 8], op=ALU.subtract)
                sa = wk.tile([128, J], F32, tag="sa", name="sa")
                nc.vector.tensor_reduce(out=sa[:],
                                        in_=dtl[:].rearrange("p (j c) -> p j c", c=8),
                                        axis=AX.X, op=ALU.add,
                                        apply_absolute_value=True)
                V = wk.tile([128, J * 8], U8, tag="V", name="V")
                nc.sync.dma_start(out=V[:], in_=vmask[s][:, j0 * 8:(j0 + J) * 8])
                nv = wk.tile([128, J], F32, tag="nv", name="nv")
                nc.vector.tensor_reduce(out=nv[:],
                                        in_=V[:].rearrange("p (j c) -> p j c", c=8),
                                        axis=AX.X, op=ALU.add)
                q = wk.tile([128, J], F32, tag="q", name="q")
                nc.vector.tensor_scalar(out=q[:], in0=nv[:], scalar1=-1.0,
                                        scalar2=7.5, op0=ALU.mult, op1=ALU.add)
                nc.vector.tensor_scalar(out=q[:], in0=q[:], scalar1=BIG,
                                        scalar2=BIG, op0=ALU.mult, op1=ALU.mult)
                O = wk.tile([128, J], F32, tag="O", name="O")
                nc.vector.tensor_tensor(out=O[:], in0=sa[:], in1=q[:], op=ALU.max)
                nc.sync.dma_start(out=out[s][:, j0:j0 + J], in_=O[:])

            # 2-deep software pipeline: slab for k+2 built before consuming k
            chunks = [(s, cch) for s in range(S) for cch in range(NSUB)]
            fifo = [build_prep(*chunks[0]), build_prep(*chunks[1])]
            for i, (s, cch) in enumerate(chunks):
                cur = fifo.pop(0)
                if i + 2 < len(chunks):
                    fifo.append(build_prep(*chunks[i + 2]))
                consume(s, cch, *cur)

    nc.compile()
    return nc


def _get_module():
    global _BUILT
    if _BUILT is None:
        _BUILT = _build_module()
    return _BUILT


def _build_t2(fr):
    """fr: [C, H, W] f32 -> [NTAB, RW] fp16 table."""
    t = np.ascontiguousarray(fr.transpose(1, 2, 0))      # [H, W, C]
    P = np.zeros((2, ROWS, 422, 8), np.float32)
    P[0, 1:129, 1:417] = t
    P[1, 0:128, 1:417] = t
    T2 = np.zeros((ROWS, WINS, RW), np.float16)
    view = T2[:, :, :EF].reshape(ROWS, WINS, 8, 3, 2)
    for d in range(3):
        # view[yi, w, c, d, pl] = P[pl, yi, 2w+d, c]
        view[:, :, :, d, :] = P[:, :, d:d + 420:2][:, :, :WINS].transpose(1, 2, 3, 0)
    return T2.reshape(NTAB, RW)


def kernel(feat_l, feat_r, lut, valid_mask, no_steps=16):
    from concourse.bass_utils import run_bass_kernel_spmd

    feat_l = np.asarray(feat_l, dtype=np.float32)
    feat_r = np.asarray(feat_r, dtype=np.float32)
    lut = np.asarray(lut, dtype=np.float32)
    vm = np.asarray(valid_mask)

    nc = _get_module()

    idm = np.eye(128, dtype=np.float32)
    repm = np.zeros((16, 128), np.float32)
    for m in range(128):
        repm[m % 16, m] = 1.0

    in_maps = []
    for b in range(B):
        in_maps.append({
            "t2": _build_t2(feat_r[b]),
            "lut3": np.ascontiguousarray(lut[b].reshape(S, H, W * 3)),
            "vmask": np.ascontiguousarray(
                vm[b].transpose(1, 2, 3, 0).astype(np.uint8).reshape(S, H, W * 8)),
            "fl": np.ascontiguousarray(
                feat_l[b].transpose(1, 2, 0).reshape(H, W * 8)),
            "iden": idm,
            "rep16": repm,
        })

    res = run_bass_kernel_spmd(nc, in_maps, core_ids=list(range(8)))
    outv = np.stack([res.results[b]["out"] for b in range(B)])
    return outv.astype(np.float32)


# revision 27
# speedup vs baseline: 1.3466x; 1.0227x over previous
"""AnyNet _build_volume_2d cost-volume kernel for 8 TRN2 NeuronCores.

Strategy: data-parallel over batch B=8 (one sample per core).  The bilinear
grid-sample gather runs on the SWDGE `dma_gather` path: one 96B descriptor
per output point fetches a 3-column x 2-row-plane x 8-channel fp16 window of
a host-prepacked table, so all 4 bilinear taps for all 8 channels arrive
with a single int16 index.  Interpolation weights are hat functions
evaluated on-device; zero padding baked into the table plus the hat
formulation reproduces grid_sample's zeros padding exactly.  The masked-inf
channel sum is computed as max(sum_c |diff_c|, +-inf) where the +-inf flag
comes from a u8 mask reduce (avoids 0*inf NaNs).

Table layout (built on host from feat_r), fp16:
  T2[yi, w, d, pl, c] = padded_feat[pl][yi][2w + d][c]   (d in [0,3), rest pad)
  padded_feat[0][yi][xc][c] = feat_r[c, yi-1, xc-1]   (zeros out of range)
  padded_feat[1][yi][xc][c] = feat_r[c, yi,   xc-1]
Rows are padded to 128 fp16 = 256B (dma_gather stride granule); the gather
fetches the first 48 fp16 (96B).  Gather index = yi*210 + w, w = xi>>1
(2-column phase windows), fits int16.

The wrapped-16 int16 index slab dma_gather requires is built with
TensorEngine permutation matmuls (engines cannot cross partitions), then
replicated to all 8 16-partition groups with a second matmul.
"""

import sys

sys.path.insert(0, "/opt/trn_rl_repo")

import numpy as np

B, C, S, H, W = 8, 8, 16, 128, 416
ROWS, WINS = 129, 210          # table rows (H+1), windows per row (W/2+2)
NTAB = ROWS * WINS             # 27090
RW = 128                       # fp16 per table row (256B stride)
EF = 48                        # fp16 fetched per index (96B: 3 cols x 16)
J = 208                        # points per partition per sub-chunk
NSUB = W // J                  # 2 sub-chunks per s-step
GIDX = 3328                    # per gather (single_packet=False lifts 1024 cap)
NGS = (128 * J) // GIDX        # 26 gathers per sub-chunk

_BUILT = None


def _raw_gather(nc, out_ap, in_ap, idxs_ap, num_idxs, elem_size, elem_step, queue_num=0):
    """dma_gather with elem_size below the wrapper's 256B floor (HW-verified)."""
    from concourse import mybir
    gp = nc.gpsimd
    stride_bytes = elem_step * mybir.dt.size(in_ap.dtype)
    _in_ap = gp.lower_ap_dma(in_ap, for_custom_bir_dma=True)
    return gp.add_instruction(
        mybir.InstDMAGatherAnt(
            name=nc.get_next_instruction_name(),
            ins=[*_in_ap, gp.lower_ap(idxs_ap),
                 gp.lower_val_access(gp.to_reg(num_idxs))],
            outs=[gp.lower_ap(out_ap)],
            transpose=False, num_idxs=num_idxs, elem_size=elem_size,
            stride_bytes_256=stride_bytes // 256, gen_mode=0,
            single_packet=False, queue_num=queue_num,
            sbuf_tokens_per_rank=0, sbuf_free_dim_per_rank=0,
            sbuf_free_dim_pad_per_rank=0, sbuf_byte_offset=0))


def _build_module():
    import concourse.bacc as bacc
    import concourse.tile as tile
    from concourse import mybir

    F32, F16, I16, U8 = (mybir.dt.float32, mybir.dt.float16, mybir.dt.int16,
                         mybir.dt.uint8)
    AF = mybir.ActivationFunctionType
    ALU = mybir.AluOpType
    AX = mybir.AxisListType

    nc = bacc.Bacc("TRN2", target_bir_lowering=False, debug=False,
                   num_devices=8, num_swdge_queues=2)
    t2 = nc.declare_dram_parameter("t2", [NTAB, RW], F16, isOutput=False)
    lut3 = nc.declare_dram_parameter("lut3", [S, 128, W * 3], F32, isOutput=False)
    vmask = nc.declare_dram_parameter("vmask", [S, 128, W * 8], U8, isOutput=False)
    fl_in = nc.declare_dram_parameter("fl", [128, W * 8], F32, isOutput=False)
    iden_in = nc.declare_dram_parameter("iden", [128, 128], F32, isOutput=False)
    rep_in = nc.declare_dram_parameter("rep16", [16, 128], F32, isOutput=False)
    out = nc.declare_dram_parameter("out", [S, 128, W], F32, isOutput=True)

    TWO23 = float(2.0 ** 23)
    BIG = 3.0e38

    with tile.TileContext(nc) as tc:
        with tc.tile_pool(name="const", bufs=1) as cst, \
             tc.tile_pool(name="work", bufs=2) as wk, \
             tc.tile_pool(name="gat", bufs=10) as gp_, \
             tc.tile_pool(name="slb", bufs=3) as slb, \
             tc.tile_pool(name="psel", bufs=1, space="PSUM") as psel, \
             tc.tile_pool(name="prep", bufs=1, space="PSUM") as prep:

            fl = cst.tile([128, W * 8], F32, tag="fl")
            nc.sync.dma_start(out=fl[:], in_=fl_in[:])
            iden = cst.tile([128, 128], F32, tag="iden")
            nc.sync.dma_start(out=iden[:], in_=iden_in[:])
            rep16 = cst.tile([16, 128], F32, tag="rep16")
            nc.sync.dma_start(out=rep16[:], in_=rep_in[:])
            cm1 = cst.tile([128, 1], F32, tag="cm1")
            nc.vector.memset(cm1[:], -1.0)
            cm2 = cst.tile([128, 1], F32, tag="cm2")
            nc.vector.memset(cm2[:], -2.0)

            def build_prep(s, cch):
                j0 = cch * J
                # ---- lut load + coordinate prep ----
                Lt = wk.tile([128, J * 3], F32, tag="Lt", name="Lt")
                nc.sync.dma_start(out=Lt[:], in_=lut3[s][:, j0 * 3:(j0 + J) * 3])
                lv = Lt[:].rearrange("p (j k) -> p j k", k=3)
                gx, gy = lv[:, :, 0], lv[:, :, 1]

                ixc = wk.tile([128, J], F32, tag="ixc", name="ixc")
                nc.scalar.activation(ixc[:], gx, AF.Copy, bias=208.5, scale=208.0)
                iyr = wk.tile([128, J], F32, tag="iyr", name="iyr")
                nc.scalar.activation(iyr[:], gy, AF.Copy, bias=64.5, scale=64.0)

                tcl = wk.tile([128, J], F32, tag="tcl", name="tcl")
                nc.vector.tensor_scalar(out=tcl[:], in0=ixc[:], scalar1=0.0,
                                        scalar2=416.9, op0=ALU.max, op1=ALU.min)
                th = wk.tile([128, J], F32, tag="th", name="th")
                nc.scalar.activation(th[:], tcl[:], AF.Copy, bias=TWO23, scale=0.5)
                rt = wk.tile([128, J], F32, tag="rt", name="rt")
                nc.scalar.activation(rt[:], th[:], AF.Copy, bias=-TWO23, scale=1.0)
                t2_ = wk.tile([128, J], F32, tag="t2_", name="t2_")
                nc.scalar.activation(t2_[:], tcl[:], AF.Copy, bias=0.0, scale=0.5)
                g1 = wk.tile([128, J], F32, tag="g1", name="g1")
                nc.vector.tensor_tensor(out=g1[:], in0=rt[:], in1=t2_[:], op=ALU.is_gt)
                wf = wk.tile([128, J], F32, tag="wf", name="wf")
                nc.vector.tensor_tensor(out=wf[:], in0=rt[:], in1=g1[:], op=ALU.subtract)
                w2 = wk.tile([128, J], F32, tag="w2", name="w2")
                nc.scalar.activation(w2[:], wf[:], AF.Copy, bias=0.0, scale=-2.0)
                u = wk.tile([128, J], F32, tag="u", name="u")
                nc.vector.tensor_tensor(out=u[:], in0=ixc[:], in1=w2[:], op=ALU.add)

                tyc = wk.tile([128, J], F32, tag="tyc", name="tyc")
                nc.vector.tensor_scalar(out=tyc[:], in0=iyr[:], scalar1=0.0,
                                        scalar2=128.9, op0=ALU.max, op1=ALU.min)
                rty = wk.tile([128, J], F32, tag="rty", name="rty")
                nc.vector.tensor_scalar(out=rty[:], in0=tyc[:], scalar1=TWO23,
                                        scalar2=-TWO23, op0=ALU.add, op1=ALU.add)
                gy1 = wk.tile([128, J], F32, tag="gy1", name="gy1")
                nc.vector.tensor_tensor(out=gy1[:], in0=rty[:], in1=tyc[:], op=ALU.is_gt)
                yi = wk.tile([128, J], F32, tag="yi", name="yi")
                nc.vector.tensor_tensor(out=yi[:], in0=rty[:], in1=gy1[:], op=ALU.subtract)
                vv = wk.tile([128, J], F32, tag="vv", name="vv")
                nc.vector.tensor_tensor(out=vv[:], in0=iyr[:], in1=yi[:], op=ALU.subtract)
                y210 = wk.tile([128, J], F32, tag="y210", name="y210")
                nc.scalar.activation(y210[:], yi[:], AF.Copy, bias=0.0, scale=210.0)
                idxf = wk.tile([128, J], F32, tag="idxf", name="idxf")
                nc.vector.tensor_tensor(out=idxf[:], in0=y210[:], in1=wf[:], op=ALU.add)

                # ---- hat weights (fp16) ----
                wx = []
                for d in range(3):
                    a = wk.tile([128, J], F32, tag=f"a{d}", name=f"a{d}")
                    bias = [0.0, cm1[:], cm2[:]][d]
                    nc.scalar.activation(a[:], u[:], AF.Abs, bias=bias, scale=1.0)
                    h = wk.tile([128, J], F16, tag=f"wx{d}", name=f"wx{d}")
                    nc.scalar.activation(h[:], a[:], AF.Relu, bias=1.0, scale=-1.0)
                    wx.append(h)
                wy = []
                for p in range(2):
                    a = wk.tile([128, J], F32, tag=f"ay{p}", name=f"ay{p}")
                    bias = [0.0, cm1[:]][p]
                    nc.scalar.activation(a[:], vv[:], AF.Abs, bias=bias, scale=1.0)
                    h = wk.tile([128, J], F16, tag=f"wy{p}", name=f"wy{p}")
                    nc.scalar.activation(h[:], a[:], AF.Relu, bias=1.0, scale=-1.0)
                    wy.append(h)
                W6 = slb.tile([128, J * 6], F16, tag="W6", name="W6")
                w6v = W6[:].rearrange("p (j k) -> p j k", k=6)
                for d in range(3):
                    for p in range(2):
                        nc.vector.tensor_tensor(out=w6v[:, :, d * 2 + p],
                                                in0=wx[d][:], in1=wy[p][:], op=ALU.mult)

                # ---- index slab via PE permutation; replicate via DMA ----
                s16 = slb.tile([16, J * 8], I16, tag="s16", name="s16")
                sv = s16[:].rearrange("q (j g) -> q j g", g=8)
                sel_ps = [psel.tile([16, 512], F32, tag=f"sel{h_}", name=f"sel{h_}") for h_ in range(4)]
                for g in range(8):
                    ps, off = sel_ps[g // 2], (g % 2) * J
                    nc.tensor.matmul(ps[:, off:off + J], iden[:, 16 * g:16 * g + 16],
                                     idxf[:], start=True, stop=True)
                for h_ in range(4):
                    pv_ = sel_ps[h_][:, 0:2 * J].rearrange("q (g j) -> q j g", g=2)
                    nc.scalar.copy(sv[:, :, 2 * h_:2 * h_ + 2], pv_)
                slab16 = slb.tile([128, J * 8], I16, tag="slab16", name="slab16")
                for g in range(8):
                    nc.sync.dma_start(out=slab16[16 * g:16 * g + 16, :], in_=s16[:])
                return slab16, w6v

            def consume(s, cch, slab16, w6v):
                j0 = cch * J
                warped = wk.tile([128, J * 8], F32, tag="warped", name="warped")
                SL = GIDX // 128   # slots per gather
                for k in range(NGS):
                    G = gp_.tile([128, SL * EF], F16, tag="G", name="G")
                    _raw_gather(nc, G[:].rearrange("p (s e) -> p s e", e=EF),
                                t2[:][:, 0:EF],
                                slab16[:, k * (GIDX // 16):(k + 1) * (GIDX // 16)],
                                GIDX, elem_size=EF, elem_step=RW, queue_num=k % 2)
                    gv = G[:].rearrange("p (j c k) -> p j c k", c=8, k=6)
                    nc.vector.tensor_tensor(
                        out=gv, in0=gv,
                        in1=w6v[:, k * SL:(k + 1) * SL, :]
                            .unsqueeze(2).broadcast_to([128, SL, 8, 6]),
                        op=ALU.mult)
                    nc.vector.tensor_reduce(
                        out=warped[:, k * SL * 8:(k + 1) * SL * 8],
                        in_=gv, axis=AX.X, op=ALU.add)

                # ---- |diff|, channel sum, masked inf ----
                dtl = wk.tile([128, J * 8], F32, tag="dtl", name="dtl")
                nc.vector.tensor_tensor(out=dtl[:], in0=warped[:],
                                        in1=fl[:, j0 * 8:(j0 + J) * 8], op=ALU.subtract)
                sa = wk.tile([128, J], F32, tag="sa", name="sa")
                nc.vector.tensor_reduce(out=sa[:],
                                        in_=dtl[:].rearrange("p (j c) -> p j c", c=8),
                                        axis=AX.X, op=ALU.add,
                                        apply_absolute_value=True)
                V = wk.tile([128, J * 8], U8, tag="V", name="V")
                nc.sync.dma_start(out=V[:], in_=vmask[s][:, j0 * 8:(j0 + J) * 8])
                nv = wk.tile([128, J], F32, tag="nv", name="nv")
                nc.vector.tensor_reduce(out=nv[:],
                                        in_=V[:].rearrange("p (j c) -> p j c", c=8),
                                        axis=AX.X, op=ALU.add)
                q = wk.tile([128, J], F32, tag="q", name="q")
                nc.vector.tensor_scalar(out=q[:], in0=nv[:], scalar1=-1.0,
                                        scalar2=7.5, op0=ALU.mult, op1=ALU.add)
                nc.vector.tensor_scalar(out=q[:], in0=q[:], scalar1=BIG,
                                        scalar2=BIG, op0=ALU.mult, op1=ALU.mult)
                O = wk.tile([128, J], F32, tag="O", name="O")
                nc.vector.tensor_tensor(out=O[:], in0=sa[:], in1=q[:], op=ALU.max)
                nc.sync.dma_start(out=out[s][:, j0:j0 + J], in_=O[:])

            # 2-deep software pipeline: slab for k+2 built before consuming k
            chunks = [(s, cch) for s in range(S) for cch in range(NSUB)]
            fifo = [build_prep(*chunks[0]), build_prep(*chunks[1])]
            for i, (s, cch) in enumerate(chunks):
                cur = fifo.pop(0)
                if i + 2 < len(chunks):
                    fifo.append(build_prep(*chunks[i + 2]))
                consume(s, cch, *cur)

    nc.compile()
    return nc


def _get_module():
    global _BUILT
    if _BUILT is None:
        _BUILT = _build_module()
    return _BUILT


def _build_t2(fr):
    """fr: [C, H, W] f32 -> [NTAB, RW] fp16 table."""
    t = np.ascontiguousarray(fr.transpose(1, 2, 0))      # [H, W, C]
    P = np.zeros((2, ROWS, 422, 8), np.float32)
    P[0, 1:129, 1:417] = t
    P[1, 0:128, 1:417] = t
    T2 = np.zeros((ROWS, WINS, RW), np.float16)
    view = T2[:, :, :EF].reshape(ROWS, WINS, 8, 3, 2)
    for d in range(3):
        # view[yi, w, c, d, pl] = P[pl, yi, 2w+d, c]
        view[:, :, :, d, :] = P[:, :, d:d + 420:2][:, :, :WINS].transpose(1, 2, 3, 0)
    return T2.reshape(NTAB, RW)


def kernel(feat_l, feat_r, lut, valid_mask, no_steps=16):
    from concourse.bass_utils import run_bass_kernel_spmd

    feat_l = np.asarray(feat_l, dtype=np.float32)
    feat_r = np.asarray(feat_r, dtype=np.float32)
    lut = np.asarray(lut, dtype=np.float32)
    vm = np.asarray(valid_mask)

    nc = _get_module()

    idm = np.eye(128, dtype=np.float32)
    repm = np.zeros((16, 128), np.float32)
    for m in range(128):
        repm[m % 16, m] = 1.0

    in_maps = []
    for b in range(B):
        in_maps.append({
            "t2": _build_t2(feat_r[b]),
            "lut3": np.ascontiguousarray(lut[b].reshape(S, H, W * 3)),
            "vmask": np.ascontiguousarray(
                vm[b].transpose(1, 2, 3, 0).astype(np.uint8).reshape(S, H, W * 8)),
            "fl": np.ascontiguousarray(
                feat_l[b].transpose(1, 2, 0).reshape(H, W * 8)),
            "iden": idm,
            "rep16": repm,
        })

    res = run_bass_kernel_spmd(nc, in_maps, core_ids=list(range(8)))
    outv = np.stack([res.results[b]["out"] for b in range(B)])
    return outv.astype(np.float32)
